# revision 7
# baseline (speedup 1.0000x reference)
"""Causal self-attention (B=1, T=2048, E=2048, 16 heads, RoPE) on 8 TRN2 NeuronCores.

Strategy: tensor-parallel over heads (2 heads/core). Each core computes
QKV for its heads, RoPE, causal softmax attention, and a PARTIAL output
projection over its 256 contraction columns of w_proj. The host sums the
8 partial [T, E] outputs (no on-device collectives).

All matmul operands are bfloat16 (fp32 PSUM accumulation, ~5e-3 max-rel
vs the 2e-2 gate). bf16 weights enable the PE's Fast Weight Load path,
halve HBM traffic, and unlock the DVE fast 16-bit modes.

Device-side layout tricks:
  - x is passed transposed: xT [E, T] so QKV matmuls contract over partitions.
  - q/k weight rows are pre-permuted per head to [even dims | odd dims] so
    RoPE becomes half-wise multiplies with cos/sin tables; the softmax scale
    1/sqrt(D) is folded into the q weights.
  - RoPE: PSUM q/k tiles are cast to bf16 once on the Scalar engine; the
    element-wise ops then run on DVE in its fast 16-bit mode.
  - scores are computed transposed: S^T[kt, qt] = kT.T @ qT, so the softmax
    sum is a ones-vector matmul and P^T feeds the V matmul directly, and
    out^T [d, qt] is exactly the lhsT the projection needs.
  - causal: only lower-triangle kt-tiles computed, and the 4 diagonal-band
    tiles are TRIMMED to their live column range [128*o, 512); one [128,128]
    0/1 triangle mask handles the残 intra-tile triangle (in-place multiply).
  - score tiles for full (non-diagonal) kt pairs share one 2-bank PSUM tile
    so exp runs once per pair (halves ACT instruction count), and the h0
    AV/row-sum matmuls are interleaved into h1's score stream so ACT's exp
    latency is hidden behind PE work.
  - normalization deferred: out^T = V^T P^T is scaled by broadcast(1/rowsum)
    past the AV matmul; 1/rowsum uses DVE reciprocal_approx_fast.
  - output partials are written in bf16 (host sums in fp32).
"""
import sys

for _p in ("/opt/trn_rl_repo",):
    if _p not in sys.path:
        sys.path.append(_p)

import numpy as np
import ml_dtypes

BF = ml_dtypes.bfloat16

B, T, E = 1, 2048, 2048
H, D = 16, 128
N_CORES = 8
HPC = H // N_CORES          # heads per core
CL = HPC * D                # contraction columns per core (256)
QC = 512                    # qt chunk (PSUM bank width in fp32)
BASE = 10000.0

_CACHE: dict = {}


# ---------------------------------------------------------------- device build
def _build_nc(t=T, debug_stop=None):
    import concourse.tile as tile
    from concourse import bacc, mybir
    from contextlib import ExitStack

    f32 = mybir.dt.float32
    bf16 = mybir.dt.bfloat16
    nj = t // QC            # qt chunks
    ntt = t // 128          # t tiles
    nct = E // 128          # contraction tiles

    nc = bacc.Bacc("TRN2", target_bir_lowering=False, debug=False,
                   enable_asserts=False, num_devices=N_CORES)
    xT_d = nc.dram_tensor("xT", [t // QC, 2, E // 256, 128, QC], bf16,
                         kind="ExternalInput").ap()
    wqkvT_d = nc.dram_tensor("wqkvT", [E, 6 * 128], bf16, kind="ExternalInput").ap()
    wprojT_d = nc.dram_tensor("wprojT", [CL, E], bf16, kind="ExternalInput").ap()
    cos2_d = nc.dram_tensor("cos2", [128, t], bf16, kind="ExternalInput").ap()
    sin2_d = nc.dram_tensor("sin2", [128, t], bf16, kind="ExternalInput").ap()
    mask_d = nc.dram_tensor("mask", [128, 128], bf16, kind="ExternalInput").ap()
    out_d = nc.dram_tensor("out", [t, E], bf16, kind="ExternalOutput").ap()

    Exp = mybir.ActivationFunctionType.Exp

    with tile.TileContext(nc) as tc:
        with ExitStack() as per:  # persistent pools
            const = per.enter_context(tc.tile_pool(name="const", bufs=1))
            wpp = per.enter_context(tc.tile_pool(name="wpp", bufs=1))
            qkp = per.enter_context(tc.tile_pool(name="qkp", bufs=1))
            vp = per.enter_context(tc.tile_pool(name="vp", bufs=1))
            atp = per.enter_context(tc.tile_pool(name="atp", bufs=1))
            ps = per.enter_context(tc.tile_pool(name="ps", bufs=1, space="PSUM"))

            # ones vectors (bf16 operands keep every matmul on the fast path)
            ones_col_f = const.tile([128, 1], f32)
            nc.vector.memset(ones_col_f[:], 1.0)
            ones_col = const.tile([128, 1], bf16)
            nc.vector.tensor_copy(ones_col[:], ones_col_f[:])
            ones_row_f = const.tile([1, 128], f32)
            nc.vector.memset(ones_row_f[:], 1.0)
            ones_row = const.tile([1, 128], bf16)
            nc.vector.tensor_copy(ones_row[:], ones_row_f[:])
            mask_sb = const.tile([128, 128], bf16)

            # persistent activations
            qk_sb = [qkp.tile([128, t], bf16, tag=f"qk{f}", name=f"qk{f}")
                     for f in range(4)]
            v_sb = vp.tile([128, ntt * 2 * D], bf16)   # [t-tile major, 2 heads*D]
            attn_sb = [atp.tile([128, t], bf16, tag=f"at{h}", name=f"at{h}")
                       for h in range(HPC)]

            # ---------------- Phase B: QKV + RoPE ----------------
            with ExitStack() as phB:
                wqp = phB.enter_context(tc.tile_pool(name="wqp", bufs=1))
                xtr = phB.enter_context(tc.tile_pool(name="xtr", bufs=3))
                rtmp = phB.enter_context(tc.tile_pool(name="rtmp", bufs=2))
                cstp = phB.enter_context(tc.tile_pool(name="cstp", bufs=1))


                # weights on the scalar-engine DMA queue, x tiles on sync,
                # constants on gpsimd -- parallel queues, compute starts asap
                wq_sb = []
                for c in range(nct):
                    w = wqp.tile([128, 6 * 128], bf16, tag=f"w{c}", name=f"w{c}")
                    eng = nc.scalar if c == 0 else nc.gpsimd
                    eng.dma_start(out=w[:], in_=wqkvT_d[c * 128:(c + 1) * 128, :])
                    wq_sb.append(w)
                # prefetch phase-C constants on the gpsimd queue (idle later)
                nc.gpsimd.dma_start(out=mask_sb[:], in_=mask_d[:])
                wp_sb = []
                for hh in range(HPC):
                    w = wpp.tile([128, E], bf16, tag=f"wp{hh}", name=f"wp{hh}")
                    nc.gpsimd.dma_start(
                        out=w[:], in_=wprojT_d[hh * 128:(hh + 1) * 128, :])
                    wp_sb.append(w)
                cos2_sb = cstp.tile([128, t], bf16)
                sin2_sb = cstp.tile([128, t], bf16)

                nhalf = nct // 2
                for j in range(nj):
                    jsl = slice(j * QC, (j + 1) * QC)
                    xts = []
                    for q in range(2):
                        xh = xtr.tile([128, nhalf * QC], bf16, tag="xt",
                                      name=f"xt{j}_{q}")
                        eng = nc.sync if q == 0 else nc.scalar
                        if j == 0:
                            # per-tile DMAs: first data reaches PE sooner
                            for cc in range(nhalf):
                                eng.dma_start(
                                    out=xh[:, cc * QC:(cc + 1) * QC],
                                    in_=xT_d[j, q, cc])
                        else:
                            eng.dma_start(
                                out=xh[:].rearrange("p (c b) -> p c b", c=nhalf),
                                in_=xT_d[j, q].rearrange("c p b -> p c b"))
                        for cc in range(nhalf):
                            xts.append(xh[:, cc * QC:(cc + 1) * QC])
                    if j == 0:
                        # rope tables after the first chunk's x tiles
                        nc.sync.dma_start(out=cos2_sb[:], in_=cos2_d[:])
                        nc.scalar.dma_start(out=sin2_sb[:], in_=sin2_d[:])
                    # q (f=0,1) and k (f=2,3), output transposed [d, t]
                    for f in range(4):
                        pq = ps.tile([128, 2 * QC], f32, tag="a", bufs=2,
                                     name=f"pq{j}_{f}")
                        for c in range(nct):
                            nc.tensor.matmul(pq[:, 0:QC],
                                             wq_sb[c][:, f * 128:(f + 1) * 128],
                                             xts[c], start=(c == 0),
                                             stop=(c == nct - 1))
                        # RoPE: out = pq*cos2 + swap_halves(pq)*sin2, sin2=[-sin; sin]
                        # PSUM->bf16 cast on ACT, then 16-bit DVE ops
                        pqb = rtmp.tile([128, QC], bf16, tag="pqb", name=f"pqb{j}_{f}")
                        nc.scalar.copy(pqb[:], pq[:, 0:QC])
                        sw = rtmp.tile([128, QC], bf16, tag="sw", name=f"sw{j}_{f}")
                        nc.vector.tensor_copy(sw[0:64, :], pqb[64:128, :])
                        nc.vector.tensor_copy(sw[64:128, :], pqb[0:64, :])
                        tB = rtmp.tile([128, QC], bf16, tag="tB", name=f"tB{j}_{f}")
                        nc.vector.tensor_mul(tB[:], sw[:], sin2_sb[:, jsl])
                        tA = rtmp.tile([128, QC], bf16, tag="tA", name=f"tA{j}_{f}")
                        nc.vector.tensor_mul(tA[:], pqb[:], cos2_sb[:, jsl])
                        nc.vector.tensor_add(qk_sb[f][:, jsl], tA[:], tB[:])
                    # v natural [t, d]: pairs of 128-t-tiles share one PSUM bank
                    for tp in range(2):
                        pv = ps.tile([128, 2 * 2 * D], f32, tag="c", bufs=3,
                                     name=f"pv{j}_{tp}")
                        for sub in range(2):
                            tt = 2 * tp + sub
                            for c in range(nct):
                                nc.tensor.matmul(
                                    pv[:, sub * 2 * D:(sub + 1) * 2 * D],
                                    xts[c][:, tt * 128:(tt + 1) * 128],
                                    wq_sb[c][:, 4 * 128:6 * 128],
                                    start=(c == 0), stop=(c == nct - 1))
                        g = j * 4 + 2 * tp
                        if tp % 2 == 0:
                            nc.scalar.copy(
                                v_sb[:, g * 2 * D:(g + 2) * 2 * D], pv[:])
                        else:
                            nc.vector.tensor_copy(
                                v_sb[:, g * 2 * D:(g + 2) * 2 * D], pv[:])

            if debug_stop == "B":
                with ExitStack() as phX:
                    outp = phX.enter_context(tc.tile_pool(name="outp", bufs=2))
                    for f in range(4):
                        ob = outp.tile([128, t], bf16, tag="ob", name=f"obB{f}")
                        nc.vector.tensor_copy(ob[:], qk_sb[f][:])
                        nc.sync.dma_start(out=out_d[f * 128:(f + 1) * 128, 0:t],
                                          in_=ob[:])

            # ---------------- Phase C: causal attention ----------------
            if debug_stop not in ("B",):
                with ExitStack() as phC:
                    ptp = phC.enter_context(tc.tile_pool(name="ptp", bufs=24))
                    ctmp = phC.enter_context(tc.tile_pool(name="ctmp", bufs=3))

                    outp = phC.enter_context(tc.tile_pool(name="outp", bufs=3))

                    def emit_proj(jj):
                        # projection + output DMA for chunk jj (overlaps later
                        # attention chunks). oc pairs share one 2-bank PSUM
                        # tile -> one DVE cast per pair.
                        for tt in range(4 * jj, 4 * jj + 4):
                            ob = outp.tile([128, E], bf16, tag="ob", name=f"obD{tt}")
                            for op in range(2):
                                pp = ps.tile([128, 2 * QC], f32, tag="a", bufs=2,
                                             name=f"pp{tt}_{op}")
                                for sub in range(2):
                                    oc = 2 * op + sub
                                    for h in range(HPC):
                                        nc.tensor.matmul(
                                            pp[:, sub * QC:(sub + 1) * QC],
                                            attn_sb[h][:, tt * 128:(tt + 1) * 128],
                                            wp_sb[h][:, oc * 512:(oc + 1) * 512],
                                            start=(h == 0), stop=(h == HPC - 1))
                                nc.vector.tensor_copy(
                                    ob[:, op * 2 * QC:(op + 1) * 2 * QC], pp[:])
                            eng = nc.sync if tt % 2 == 0 else nc.scalar
                            eng.dma_start(out=out_d[tt * 128:(tt + 1) * 128, :],
                                          in_=ob[:])

                    def flush_norm(pending):
                        # deferred normalization tail: bc matmul overlaps the
                        # NEXT chunk's S^T stream instead of stalling PE
                        for (jj, h, po_, inv_) in pending:
                            jjsl = slice(jj * QC, (jj + 1) * QC)
                            invb = ctmp.tile([1, QC], bf16, tag="invb", bufs=6,
                                             name=f"invb{jj}_{h}")
                            nc.vector.tensor_copy(invb[:], inv_[:])
                            bc = ps.tile([128, 2 * QC], f32, tag="a", bufs=2,
                                         name=f"bc{jj}_{h}")
                            nc.tensor.matmul(bc[:, 0:QC], ones_row[:], invb[:],
                                             start=True, stop=True)
                            bcs = ctmp.tile([128, QC], bf16, tag="bcs",
                                            name=f"bcs{jj}_{h}")
                            nc.vector.tensor_copy(bcs[:], bc[:, 0:QC])
                            nc.vector.tensor_mul(attn_sb[h][:, jjsl], po_[:], bcs[:])

                    def emit_scores(j, h, pts):
                        # S^T tiles + exp for head h of chunk j. Full kt tiles
                        # are computed in PAIRS sharing a 2-bank PSUM tile
                        # (one exp per pair); the 4 diagonal tiles are trimmed
                        # to live columns [128o, 512) and masked in place.
                        nkt = 4 * (j + 1)
                        jq0 = j * QC
                        for kp in range(2 * j):      # full-tile pairs
                            stp = ps.tile([128, 2 * QC], f32, tag="a", bufs=2,
                                          name=f"st{j}_{h}_{kp}")
                            for sub in range(2):
                                k = 2 * kp + sub
                                nc.tensor.matmul(
                                    stp[:, sub * QC:(sub + 1) * QC],
                                    qk_sb[2 + h][:, k * 128:(k + 1) * 128],
                                    qk_sb[h][:, jq0:jq0 + QC],
                                    start=True, stop=True)
                            pt = ptp.tile([128, 2 * QC], bf16, tag="pt",
                                          name=f"pt{j}_{h}_{kp}")
                            nc.scalar.activation(pt[:], stp[:], Exp)
                            pts[(h, 2 * kp)] = pt[:, 0:QC]
                            pts[(h, 2 * kp + 1)] = pt[:, QC:2 * QC]
                        for o in range(4):           # diagonal band, trimmed
                            k = 4 * j + o
                            off = 128 * o
                            stp = ps.tile([128, 2 * QC], f32, tag="a", bufs=2,
                                          name=f"st{j}_{h}_d{o}")
                            nc.tensor.matmul(
                                stp[:, off:QC],
                                qk_sb[2 + h][:, k * 128:(k + 1) * 128],
                                qk_sb[h][:, jq0 + off:jq0 + QC],
                                start=True, stop=True)
                            pt = ptp.tile([128, QC], bf16, tag="ptd",
                                          name=f"ptd{j}_{h}_{o}")
                            nc.scalar.activation(pt[:, off:QC], stp[:, off:QC],
                                                 Exp)
                            nc.vector.tensor_mul(pt[:, off:off + 128],
                                                 pt[:, off:off + 128],
                                                 mask_sb[:])
                            pts[(h, k)] = pt[:, 0:QC]

                    def po_ss_step(j, h, k, po, ss, pts, nkt):
                        off = 128 * (k - 4 * j) if k >= 4 * j else 0
                        nc.tensor.matmul(po[:, off:QC],
                                         v_sb[:, k * 2 * D + h * D:
                                              k * 2 * D + (h + 1) * D],
                                         pts[(h, k)][:, off:QC],
                                         start=(k == 0), stop=(k == nkt - 1))
                        nc.tensor.matmul(ss[:, off:QC], ones_col[:],
                                         pts[(h, k)][:, off:QC],
                                         start=(k == 0), stop=(k == nkt - 1))

                    pending = []
                    for j in range(nj):
                        nkt = 4 * (j + 1)
                        pts = {}
                        emit_scores(j, 0, pts)
                        # previous chunk's normalization: bc matmuls overlap
                        # this chunk's exp backlog
                        flush_norm(pending)
                        pending = []
                        # h1 scores interleaved with h0 AV/row-sum: gives ACT
                        # a full S-stream of lead time on every exp
                        po0 = ps.tile([128, QC], f32, tag="c", bufs=3,
                                      name=f"po{j}_0")
                        ss0 = ps.tile([1, QC], f32, tag="b", bufs=1,
                                      name=f"ss{j}_0")
                        emit1 = []
                        for kp in range(2 * j):
                            emit1.append(("pair", kp))
                        for o in range(4):
                            emit1.append(("diag", o))
                        k0 = 0

                        def h1_unit(unit):
                            kind, idx = unit
                            if kind == "pair":
                                kp = idx
                                stp = ps.tile([128, 2 * QC], f32, tag="a",
                                              bufs=2, name=f"st{j}_1_{kp}")
                                for sub in range(2):
                                    k = 2 * kp + sub
                                    nc.tensor.matmul(
                                        stp[:, sub * QC:(sub + 1) * QC],
                                        qk_sb[3][:, k * 128:(k + 1) * 128],
                                        qk_sb[1][:, j * QC:(j + 1) * QC],
                                        start=True, stop=True)
                                pt = ptp.tile([128, 2 * QC], bf16, tag="pt",
                                              name=f"pt{j}_1_{kp}")
                                nc.scalar.activation(pt[:], stp[:], Exp)
                                pts[(1, 2 * kp)] = pt[:, 0:QC]
                                pts[(1, 2 * kp + 1)] = pt[:, QC:2 * QC]
                            else:
                                o = idx
                                k = 4 * j + o
                                off = 128 * o
                                stp = ps.tile([128, 2 * QC], f32, tag="a",
                                              bufs=2, name=f"st{j}_1_d{o}")
                                nc.tensor.matmul(
                                    stp[:, off:QC],
                                    qk_sb[3][:, k * 128:(k + 1) * 128],
                                    qk_sb[1][:, j * QC + off:(j + 1) * QC],
                                    start=True, stop=True)
                                pt = ptp.tile([128, QC], bf16, tag="ptd",
                                              name=f"ptd{j}_1_{o}")
                                nc.scalar.activation(pt[:, off:QC],
                                                     stp[:, off:QC], Exp)
                                nc.vector.tensor_mul(pt[:, off:off + 128],
                                                     pt[:, off:off + 128],
                                                     mask_sb[:])
                                pts[(1, k)] = pt[:, 0:QC]

                        for unit in emit1:
                            h1_unit(unit)
                            # two h0 po/ss steps per h1 unit (h1 emits ~2
                            # tiles per unit on average)
                            for _ in range(2):
                                if k0 < nkt:
                                    po_ss_step(j, 0, k0, po0, ss0, pts, nkt)
                                    k0 += 1
                        while k0 < nkt:
                            po_ss_step(j, 0, k0, po0, ss0, pts, nkt)
                            k0 += 1
                        inv0 = ctmp.tile([1, QC], f32, tag="inv", bufs=6,
                                         name=f"inv{j}_0")
                        nc.vector.reciprocal_approx_fast(inv0[:], ss0[:])
                        pending.append((j, 0, po0, inv0))
                        if j > 0 and debug_stop is None:
                            # projection of the previous chunk keeps PE busy
                            # while ACT drains head-1 exps
                            emit_proj(j - 1)
                        po1 = ps.tile([128, QC], f32, tag="c", bufs=3,
                                      name=f"po{j}_1")
                        ss1 = ps.tile([1, QC], f32, tag="b", bufs=1,
                                      name=f"ss{j}_1")
                        for k in range(nkt):
                            po_ss_step(j, 1, k, po1, ss1, pts, nkt)
                        inv1 = ctmp.tile([1, QC], f32, tag="inv", bufs=6,
                                         name=f"inv{j}_1")
                        nc.vector.reciprocal_approx_fast(inv1[:], ss1[:])
                        pending.append((j, 1, po1, inv1))
                    flush_norm(pending)
                    if debug_stop is None:
                        emit_proj(nj - 1)

            if debug_stop == "C":
                with ExitStack() as phX:
                    outp = phX.enter_context(tc.tile_pool(name="outp", bufs=2))
                    for h in range(HPC):
                        ob = outp.tile([128, t], bf16, tag="ob", name=f"obC{h}")
                        nc.vector.tensor_copy(ob[:], attn_sb[h][:])
                        nc.sync.dma_start(out=out_d[h * 128:(h + 1) * 128, 0:t],
                                          in_=ob[:])


    nc.compile()
    return nc


# ---------------------------------------------------------------- host prep
def _rope_perm():
    p = np.empty(E, dtype=np.int64)
    for h in range(H):
        b = h * D
        p[b:b + 64] = b + np.arange(0, D, 2)
        p[b + 64:b + D] = b + np.arange(1, D, 2)
    return p


def _tables(t=T):
    # match reference: fp32 theta, fp32 angles; tables quantized to bf16
    theta = (1.0 / (BASE ** (np.arange(0, D, 2, dtype=np.float32) / np.float32(D)))
             ).astype(np.float32)
    m = np.arange(t, dtype=np.float32)
    fr = np.outer(m, theta).astype(np.float32)        # [t, 64]
    cos = np.cos(fr).T.astype(np.float32)             # [64, t]
    sin = np.sin(fr).T.astype(np.float32)
    cos2 = np.ascontiguousarray(np.concatenate([cos, cos], 0).astype(BF))
    sin2 = np.ascontiguousarray(np.concatenate([-sin, sin], 0).astype(BF))
    return cos2, sin2


def _mask():
    a = np.arange(128)[:, None]
    b = np.arange(128)[None, :]
    return np.ascontiguousarray((b >= a).astype(BF))


def _prep_inputs(x, w_attn, w_proj, t=T):
    x2 = np.asarray(x, dtype=np.float32).reshape(t, E)
    xT = np.ascontiguousarray(
        x2.T.reshape(2, E // 256, 128, t // QC, QC).transpose(3, 0, 1, 2, 4)
        .astype(BF))
    perm = _rope_perm()
    scale = np.float32(1.0) / np.sqrt(np.float32(D))
    wq = np.asarray(w_attn[0:E])[perm] * scale
    wk = np.asarray(w_attn[E:2 * E])[perm]
    wv = np.asarray(w_attn[2 * E:3 * E])
    cos2, sin2 = _tables(t)
    mask = _mask()
    in_maps = []
    for c in range(N_CORES):
        rows = slice(c * CL, (c + 1) * CL)
        wqkv = np.concatenate([wq[rows], wk[rows], wv[rows]], axis=0)  # [768, E]
        in_maps.append({
            "xT": xT,
            "wqkvT": np.ascontiguousarray(wqkv.T.astype(BF)),
            "wprojT": np.ascontiguousarray(
                np.asarray(w_proj)[:, rows].T.astype(BF)),
            "cos2": cos2,
            "sin2": sin2,
            "mask": mask,
        })
    return in_maps


# ---------------------------------------------------------------- cached runner
def _get_runner(t=T, debug_stop=None):
    """Build the Bass module once and return a cached jitted executor.

    Mirrors concourse.bass2jax.run_bass_via_pjrt's multi-core branch, but
    keeps the jitted callable so repeated kernel() calls don't recompile.
    """
    key = ("runner", t, debug_stop)
    if key in _CACHE:
        return _CACHE[key]
    import jax
    from concourse import bass2jax, mybir
    from jax.experimental.shard_map import shard_map
    from jax.sharding import Mesh, PartitionSpec

    nc = _build_nc(t, debug_stop)
    bass2jax.install_neuronx_cc_hook()

    partition_name = (nc.partition_id_tensor.name if nc.partition_id_tensor
                      else None)
    in_names, out_names, out_avals, zero_shapes = [], [], [], []
    for alloc in nc.m.functions[0].allocations:
        if not isinstance(alloc, mybir.MemoryLocationSet):
            continue
        name = alloc.memorylocations[0].name
        if alloc.kind == "ExternalInput":
            if name != partition_name:
                in_names.append(name)
        elif alloc.kind == "ExternalOutput":
            shape = tuple(alloc.tensor_shape)
            dtype = mybir.dt.np(alloc.dtype)
            out_names.append(name)
            out_avals.append(jax.core.ShapedArray(shape, dtype))
            zero_shapes.append((shape, dtype))
    n_params = len(in_names)
    all_in_names = list(in_names) + list(out_names)
    if partition_name is not None:
        all_in_names.append(partition_name)

    def _body(*args):
        operands = list(args)
        if partition_name is not None:
            operands.append(bass2jax.partition_id_tensor())
        outs = bass2jax._bass_exec_p.bind(
            *operands,
            out_avals=tuple(out_avals),
            in_names=tuple(all_in_names),
            out_names=tuple(out_names),
            lowering_input_output_aliases=(),
            sim_require_finite=True,
            sim_require_nnan=True,
            nc=nc,
        )
        return tuple(outs)

    devices = jax.devices()[:N_CORES]
    mesh = Mesh(np.asarray(devices), ("core",))
    donate = tuple(range(n_params, n_params + len(out_names)))
    sharded = jax.jit(
        shard_map(_body, mesh=mesh,
                  in_specs=(PartitionSpec("core"),) * (n_params + len(out_names)),
                  out_specs=(PartitionSpec("core"),) * len(out_names)),
        donate_argnums=donate, keep_unused=True)

    runner = {"fn": sharded, "in_names": in_names, "out_names": out_names,
              "out_avals": out_avals, "zero_shapes": zero_shapes, "nc": nc}
    _CACHE[key] = runner
    return runner


def _run(in_maps, t=T, debug_stop=None):
    r = _get_runner(t, debug_stop)
    concat_in = [
        np.concatenate([np.asarray(in_maps[c][name]) for c in range(N_CORES)],
                       axis=0)
        for name in r["in_names"]
    ]
    concat_zeros = [np.zeros((N_CORES * s[0], *s[1:]), d)
                    for (s, d) in r["zero_shapes"]]
    out_arrs = r["fn"](*concat_in, *concat_zeros)
    outs = []
    for c in range(N_CORES):
        outs.append({
            name: np.asarray(out_arrs[i]).reshape(N_CORES,
                                                  *r["out_avals"][i].shape)[c]
            for i, name in enumerate(r["out_names"])
        })
    return outs


# ---------------------------------------------------------------- entry point
def kernel(x, w_attn, w_proj):
    x = np.asarray(x, dtype=np.float32)
    w_attn = np.asarray(w_attn, dtype=np.float32)
    w_proj = np.asarray(w_proj, dtype=np.float32)
    in_maps = _prep_inputs(x, w_attn, w_proj)
    outs = _run(in_maps)
    acc = outs[0]["out"].astype(np.float32)
    for c in range(1, N_CORES):
        acc = acc + outs[c]["out"].astype(np.float32)
    return acc.reshape(B, T, E).astype(np.float32)


# revision 9
# speedup vs baseline: 1.2257x; 1.2257x over previous
"""Causal self-attention (B=1, T=2048, E=2048, 16 heads, RoPE) on 8 TRN2 NeuronCores.

Strategy: tensor-parallel over heads (2 heads/core). Each core computes
QKV for its heads, RoPE, causal softmax attention, and a PARTIAL output
projection over its 256 contraction columns of w_proj. The host sums the
8 partial [T, E] outputs (no on-device collectives).

All matmul operands are bfloat16 (fp32 PSUM accumulation, ~5e-3 max-rel
vs the 2e-2 gate). bf16 weights enable the PE's Fast Weight Load path,
halve HBM traffic, and unlock the DVE fast 16-bit modes.

Device-side layout tricks:
  - x is passed transposed: xT [E, T] so QKV matmuls contract over partitions.
  - q/k weight rows are pre-permuted per head to [even dims | odd dims] so
    RoPE becomes half-wise multiplies with cos/sin tables; the softmax scale
    1/sqrt(D) is folded into the q weights.
  - RoPE: PSUM q/k tiles are cast to bf16 once on the Scalar engine; the
    element-wise ops then run on DVE in its fast 16-bit mode.
  - scores are computed transposed: S^T[kt, qt] = kT.T @ qT, so the softmax
    sum is a ones-vector matmul and P^T feeds the V matmul directly, and
    out^T [d, qt] is exactly the lhsT the projection needs.
  - causal: only lower-triangle kt-tiles computed, and the 4 diagonal-band
    tiles are TRIMMED to their live column range [128*o, 512); one [128,128]
    0/1 triangle mask handles the残 intra-tile triangle (in-place multiply).
  - score tiles for full (non-diagonal) kt pairs share one 2-bank PSUM tile
    so exp runs once per pair (halves ACT instruction count), and the h0
    AV/row-sum matmuls are interleaved into h1's score stream so ACT's exp
    latency is hidden behind PE work.
  - normalization deferred: out^T = V^T P^T is scaled by broadcast(1/rowsum)
    past the AV matmul; 1/rowsum uses DVE reciprocal_approx_fast.
  - output partials are written in bf16 (host sums in fp32).
"""
import sys

for _p in ("/opt/trn_rl_repo",):
    if _p not in sys.path:
        sys.path.append(_p)

import numpy as np
import ml_dtypes

BF = ml_dtypes.bfloat16

B, T, E = 1, 2048, 2048
H, D = 16, 128
N_CORES = 8
HPC = H // N_CORES          # heads per core
CL = HPC * D                # contraction columns per core (256)
QC = 512                    # qt chunk (PSUM bank width in fp32)
BASE = 10000.0

_CACHE: dict = {}


# ---------------------------------------------------------------- device build
def _build_nc(t=T, debug_stop=None):
    import concourse.tile as tile
    from concourse import bacc, mybir
    from contextlib import ExitStack

    f32 = mybir.dt.float32
    bf16 = mybir.dt.bfloat16
    nj = t // QC            # qt chunks
    ntt = t // 128          # t tiles
    nct = E // 128          # contraction tiles

    nc = bacc.Bacc("TRN2", target_bir_lowering=False, debug=False,
                   enable_asserts=False, num_devices=N_CORES)
    xT_d = nc.dram_tensor("xT", [t // QC, 2, E // 256, 128, QC], bf16,
                         kind="ExternalInput").ap()
    wqkvT_d = nc.dram_tensor("wqkvT", [E, 6 * 128], bf16, kind="ExternalInput").ap()
    wprojT_d = nc.dram_tensor("wprojT", [CL, E], bf16, kind="ExternalInput").ap()
    cos2_d = nc.dram_tensor("cos2", [128, t], bf16, kind="ExternalInput").ap()
    sin2_d = nc.dram_tensor("sin2", [128, t], bf16, kind="ExternalInput").ap()
    mask_d = nc.dram_tensor("mask", [128, 128], bf16, kind="ExternalInput").ap()
    out_d = nc.dram_tensor("out", [t, E], bf16, kind="ExternalOutput").ap()

    Exp = mybir.ActivationFunctionType.Exp

    with tile.TileContext(nc) as tc:
        with ExitStack() as per:  # persistent pools
            const = per.enter_context(tc.tile_pool(name="const", bufs=1))
            wpp = per.enter_context(tc.tile_pool(name="wpp", bufs=1))
            qkp = per.enter_context(tc.tile_pool(name="qkp", bufs=1))
            vp = per.enter_context(tc.tile_pool(name="vp", bufs=1))
            atp = per.enter_context(tc.tile_pool(name="atp", bufs=1))
            ps = per.enter_context(tc.tile_pool(name="ps", bufs=1, space="PSUM"))

            # ones vectors (bf16 operands keep every matmul on the fast path)
            ones_col_f = const.tile([128, 1], f32)
            nc.vector.memset(ones_col_f[:], 1.0)
            ones_col = const.tile([128, 1], bf16)
            nc.vector.tensor_copy(ones_col[:], ones_col_f[:])
            ones_row_f = const.tile([1, 128], f32)
            nc.vector.memset(ones_row_f[:], 1.0)
            ones_row = const.tile([1, 128], bf16)
            nc.vector.tensor_copy(ones_row[:], ones_row_f[:])
            mask_sb = const.tile([128, 128], bf16)

            # persistent activations
            qk_sb = [qkp.tile([128, t], bf16, tag=f"qk{f}", name=f"qk{f}")
                     for f in range(4)]
            v_sb = vp.tile([128, ntt * 2 * D], bf16)   # [t-tile major, 2 heads*D]
            attn_sb = [atp.tile([128, t], bf16, tag=f"at{h}", name=f"at{h}")
                       for h in range(HPC)]

            # ---------------- Phase B: QKV + RoPE ----------------
            with ExitStack() as phB:
                wqp = phB.enter_context(tc.tile_pool(name="wqp", bufs=1))
                xtr = phB.enter_context(tc.tile_pool(name="xtr", bufs=3))
                rtmp = phB.enter_context(tc.tile_pool(name="rtmp", bufs=2))
                cstp = phB.enter_context(tc.tile_pool(name="cstp", bufs=1))


                # weights on the scalar-engine DMA queue, x tiles on sync,
                # constants on gpsimd -- parallel queues, compute starts asap
                wq_sb = []
                for c in range(nct):
                    w = wqp.tile([128, 6 * 128], bf16, tag=f"w{c}", name=f"w{c}")
                    eng = nc.scalar if c == 0 else nc.gpsimd
                    eng.dma_start(out=w[:], in_=wqkvT_d[c * 128:(c + 1) * 128, :])
                    wq_sb.append(w)
                # prefetch phase-C constants on the gpsimd queue (idle later)
                nc.gpsimd.dma_start(out=mask_sb[:], in_=mask_d[:])
                wp_sb = []
                for hh in range(HPC):
                    w = wpp.tile([128, E], bf16, tag=f"wp{hh}", name=f"wp{hh}")
                    nc.gpsimd.dma_start(
                        out=w[:], in_=wprojT_d[hh * 128:(hh + 1) * 128, :])
                    wp_sb.append(w)
                cos2_sb = cstp.tile([128, t], bf16)
                sin2_sb = cstp.tile([128, t], bf16)

                nhalf = nct // 2
                for j in range(nj):
                    jsl = slice(j * QC, (j + 1) * QC)
                    xts = []
                    for q in range(2):
                        xh = xtr.tile([128, nhalf * QC], bf16, tag="xt",
                                      name=f"xt{j}_{q}")
                        eng = nc.sync if q == 0 else nc.scalar
                        if j == 0:
                            # per-tile DMAs: first data reaches PE sooner
                            for cc in range(nhalf):
                                eng.dma_start(
                                    out=xh[:, cc * QC:(cc + 1) * QC],
                                    in_=xT_d[j, q, cc])
                        else:
                            eng.dma_start(
                                out=xh[:].rearrange("p (c b) -> p c b", c=nhalf),
                                in_=xT_d[j, q].rearrange("c p b -> p c b"))
                        for cc in range(nhalf):
                            xts.append(xh[:, cc * QC:(cc + 1) * QC])
                    if j == 0:
                        # rope tables after the first chunk's x tiles
                        nc.sync.dma_start(out=cos2_sb[:], in_=cos2_d[:])
                        nc.scalar.dma_start(out=sin2_sb[:], in_=sin2_d[:])
                    # q (f=0,1) and k (f=2,3), output transposed [d, t]
                    for f in range(4):
                        pq = ps.tile([128, QC], f32, tag="a", bufs=4,
                                     name=f"pq{j}_{f}")
                        for c in range(nct):
                            nc.tensor.matmul(pq[:],
                                             wq_sb[c][:, f * 128:(f + 1) * 128],
                                             xts[c], start=(c == 0),
                                             stop=(c == nct - 1))
                        # RoPE: out = pq*cos2 + swap_halves(pq)*sin2, sin2=[-sin; sin]
                        # PSUM->bf16 cast on ACT, then 16-bit DVE ops
                        pqb = rtmp.tile([128, QC], bf16, tag="pqb", name=f"pqb{j}_{f}")
                        nc.scalar.copy(pqb[:], pq[:])
                        sw = rtmp.tile([128, QC], bf16, tag="sw", name=f"sw{j}_{f}")
                        nc.vector.tensor_copy(sw[0:64, :], pqb[64:128, :])
                        nc.vector.tensor_copy(sw[64:128, :], pqb[0:64, :])
                        tB = rtmp.tile([128, QC], bf16, tag="tB", name=f"tB{j}_{f}")
                        nc.vector.tensor_mul(tB[:], sw[:], sin2_sb[:, jsl])
                        tA = rtmp.tile([128, QC], bf16, tag="tA", name=f"tA{j}_{f}")
                        nc.vector.tensor_mul(tA[:], pqb[:], cos2_sb[:, jsl])
                        nc.vector.tensor_add(qk_sb[f][:, jsl], tA[:], tB[:])
                    # v natural [t, d] per 128-t-tile
                    for tt in range(4):
                        pv = ps.tile([128, 2 * D], f32, tag="c", bufs=3,
                                     name=f"pv{j}_{tt}")
                        for c in range(nct):
                            nc.tensor.matmul(pv[:], xts[c][:, tt * 128:(tt + 1) * 128],
                                             wq_sb[c][:, 4 * 128:6 * 128],
                                             start=(c == 0), stop=(c == nct - 1))
                        g = j * 4 + tt
                        if tt % 2 == 0:
                            nc.scalar.copy(
                                v_sb[:, g * 2 * D:(g + 1) * 2 * D], pv[:])
                        else:
                            nc.vector.tensor_copy(
                                v_sb[:, g * 2 * D:(g + 1) * 2 * D], pv[:])

            if debug_stop == "B":
                with ExitStack() as phX:
                    outp = phX.enter_context(tc.tile_pool(name="outp", bufs=2))
                    for f in range(4):
                        ob = outp.tile([128, t], bf16, tag="ob", name=f"obB{f}")
                        nc.vector.tensor_copy(ob[:], qk_sb[f][:])
                        nc.sync.dma_start(out=out_d[f * 128:(f + 1) * 128, 0:t],
                                          in_=ob[:])

            # ---------------- Phase C: causal attention ----------------
            if debug_stop not in ("B",):
                with ExitStack() as phC:
                    ptp = phC.enter_context(tc.tile_pool(name="ptp", bufs=24))
                    ctmp = phC.enter_context(tc.tile_pool(name="ctmp", bufs=3))

                    outp = phC.enter_context(tc.tile_pool(name="outp", bufs=3))

                    def emit_proj(jj):
                        # projection + output DMA for chunk jj (overlaps later
                        # attention chunks). Casts go to DVE only: ACT is busy
                        # with the current chunk's exp stream.
                        for tt in range(4 * jj, 4 * jj + 4):
                            ob = outp.tile([128, E], bf16, tag="ob", name=f"obD{tt}")
                            for oc in range(E // 512):
                                pp = ps.tile([128, 512], f32, tag="a", bufs=4,
                                             name=f"pp{tt}_{oc}")
                                for h in range(HPC):
                                    nc.tensor.matmul(
                                        pp[:], attn_sb[h][:, tt * 128:(tt + 1) * 128],
                                        wp_sb[h][:, oc * 512:(oc + 1) * 512],
                                        start=(h == 0), stop=(h == HPC - 1))
                                nc.vector.tensor_copy(
                                    ob[:, oc * 512:(oc + 1) * 512], pp[:])
                            eng = nc.sync if tt % 2 == 0 else nc.scalar
                            eng.dma_start(out=out_d[tt * 128:(tt + 1) * 128, :],
                                          in_=ob[:])

                    def flush_norm(pending):
                        # deferred normalization tail: bc matmul overlaps the
                        # NEXT chunk's S^T stream instead of stalling PE
                        for (jj, h, po_, inv_) in pending:
                            jjsl = slice(jj * QC, (jj + 1) * QC)
                            invb = ctmp.tile([1, QC], bf16, tag="invb", bufs=6,
                                             name=f"invb{jj}_{h}")
                            nc.vector.tensor_copy(invb[:], inv_[:])
                            bc = ps.tile([128, QC], f32, tag="a", bufs=4,
                                         name=f"bc{jj}_{h}")
                            nc.tensor.matmul(bc[:], ones_row[:], invb[:],
                                             start=True, stop=True)
                            bcs = ctmp.tile([128, QC], bf16, tag="bcs",
                                            name=f"bcs{jj}_{h}")
                            nc.vector.tensor_copy(bcs[:], bc[:])
                            nc.vector.tensor_mul(attn_sb[h][:, jjsl], po_[:], bcs[:])

                    pending = []
                    for j in range(nj):
                        jq0 = j * QC
                        nkt = 4 * (j + 1)
                        pts = {}
                        for h in range(HPC):
                            for k in range(nkt):
                                # diagonal-band tiles trimmed to live columns
                                o = k - 4 * j
                                off = 128 * o if o > 0 else 0
                                stp = ps.tile([128, QC], f32, tag="a", bufs=4,
                                              name=f"st{j}_{h}_{k}")
                                nc.tensor.matmul(
                                    stp[:, off:QC],
                                    qk_sb[2 + h][:, k * 128:(k + 1) * 128],
                                    qk_sb[h][:, jq0 + off:jq0 + QC],
                                    start=True, stop=True)
                                pt = ptp.tile([128, QC], bf16, tag="pt",
                                              name=f"pt{j}_{h}_{k}")
                                nc.scalar.activation(pt[:, off:QC],
                                                     stp[:, off:QC], Exp)
                                if o >= 0:
                                    nc.vector.tensor_mul(pt[:, off:off + 128],
                                                         pt[:, off:off + 128],
                                                         mask_sb[:])
                                pts[(h, k)] = pt
                            if h == 0:
                                # free previous chunk's po slots mid-stream:
                                # bc matmuls still overlap this chunk's S^T
                                flush_norm(pending)
                                pending = []
                        for h in range(HPC):
                            po = ps.tile([128, QC], f32, tag="c", bufs=3,
                                         name=f"po{j}_{h}")
                            ss = ps.tile([1, QC], f32, tag="b", bufs=1,
                                         name=f"ss{j}_{h}")
                            for k in range(nkt):
                                o = k - 4 * j
                                off = 128 * o if o > 0 else 0
                                nc.tensor.matmul(
                                    po[:, off:QC],
                                    v_sb[:, k * 2 * D + h * D:
                                         k * 2 * D + (h + 1) * D],
                                    pts[(h, k)][:, off:QC],
                                    start=(k == 0), stop=(k == nkt - 1))
                                nc.tensor.matmul(
                                    ss[:, off:QC], ones_col[:],
                                    pts[(h, k)][:, off:QC],
                                    start=(k == 0), stop=(k == nkt - 1))
                            inv = ctmp.tile([1, QC], f32, tag="inv", bufs=6,
                                            name=f"inv{j}_{h}")
                            nc.vector.reciprocal_approx_fast(inv[:], ss[:])
                            pending.append((j, h, po, inv))
                            if h == 0 and j > 0 and debug_stop is None:
                                # projection of the previous chunk keeps PE busy
                                # while ACT drains head-1 exps
                                emit_proj(j - 1)
                    flush_norm(pending)
                    if debug_stop is None:
                        emit_proj(nj - 1)

            if debug_stop == "C":
                with ExitStack() as phX:
                    outp = phX.enter_context(tc.tile_pool(name="outp", bufs=2))
                    for h in range(HPC):
                        ob = outp.tile([128, t], bf16, tag="ob", name=f"obC{h}")
                        nc.vector.tensor_copy(ob[:], attn_sb[h][:])
                        nc.sync.dma_start(out=out_d[h * 128:(h + 1) * 128, 0:t],
                                          in_=ob[:])


    nc.compile()
    return nc


# ---------------------------------------------------------------- host prep
def _rope_perm():
    p = np.empty(E, dtype=np.int64)
    for h in range(H):
        b = h * D
        p[b:b + 64] = b + np.arange(0, D, 2)
        p[b + 64:b + D] = b + np.arange(1, D, 2)
    return p


def _tables(t=T):
    # match reference: fp32 theta, fp32 angles; tables quantized to bf16
    theta = (1.0 / (BASE ** (np.arange(0, D, 2, dtype=np.float32) / np.float32(D)))
             ).astype(np.float32)
    m = np.arange(t, dtype=np.float32)
    fr = np.outer(m, theta).astype(np.float32)        # [t, 64]
    cos = np.cos(fr).T.astype(np.float32)             # [64, t]
    sin = np.sin(fr).T.astype(np.float32)
    cos2 = np.ascontiguousarray(np.concatenate([cos, cos], 0).astype(BF))
    sin2 = np.ascontiguousarray(np.concatenate([-sin, sin], 0).astype(BF))
    return cos2, sin2


def _mask():
    a = np.arange(128)[:, None]
    b = np.arange(128)[None, :]
    return np.ascontiguousarray((b >= a).astype(BF))


def _prep_inputs(x, w_attn, w_proj, t=T):
    x2 = np.asarray(x, dtype=np.float32).reshape(t, E)
    xT = np.ascontiguousarray(
        x2.T.reshape(2, E // 256, 128, t // QC, QC).transpose(3, 0, 1, 2, 4)
        .astype(BF))
    perm = _rope_perm()
    scale = np.float32(1.0) / np.sqrt(np.float32(D))
    wq = np.asarray(w_attn[0:E])[perm] * scale
    wk = np.asarray(w_attn[E:2 * E])[perm]
    wv = np.asarray(w_attn[2 * E:3 * E])
    cos2, sin2 = _tables(t)
    mask = _mask()
    in_maps = []
    for c in range(N_CORES):
        rows = slice(c * CL, (c + 1) * CL)
        wqkv = np.concatenate([wq[rows], wk[rows], wv[rows]], axis=0)  # [768, E]
        in_maps.append({
            "xT": xT,
            "wqkvT": np.ascontiguousarray(wqkv.T.astype(BF)),
            "wprojT": np.ascontiguousarray(
                np.asarray(w_proj)[:, rows].T.astype(BF)),
            "cos2": cos2,
            "sin2": sin2,
            "mask": mask,
        })
    return in_maps


# ---------------------------------------------------------------- cached runner
def _get_runner(t=T, debug_stop=None):
    """Build the Bass module once and return a cached jitted executor.

    Mirrors concourse.bass2jax.run_bass_via_pjrt's multi-core branch, but
    keeps the jitted callable so repeated kernel() calls don't recompile.
    """
    key = ("runner", t, debug_stop)
    if key in _CACHE:
        return _CACHE[key]
    import jax
    from concourse import bass2jax, mybir
    from jax.experimental.shard_map import shard_map
    from jax.sharding import Mesh, PartitionSpec

    nc = _build_nc(t, debug_stop)
    bass2jax.install_neuronx_cc_hook()

    partition_name = (nc.partition_id_tensor.name if nc.partition_id_tensor
                      else None)
    in_names, out_names, out_avals, zero_shapes = [], [], [], []
    for alloc in nc.m.functions[0].allocations:
        if not isinstance(alloc, mybir.MemoryLocationSet):
            continue
        name = alloc.memorylocations[0].name
        if alloc.kind == "ExternalInput":
            if name != partition_name:
                in_names.append(name)
        elif alloc.kind == "ExternalOutput":
            shape = tuple(alloc.tensor_shape)
            dtype = mybir.dt.np(alloc.dtype)
            out_names.append(name)
            out_avals.append(jax.core.ShapedArray(shape, dtype))
            zero_shapes.append((shape, dtype))
    n_params = len(in_names)
    all_in_names = list(in_names) + list(out_names)
    if partition_name is not None:
        all_in_names.append(partition_name)

    def _body(*args):
        operands = list(args)
        if partition_name is not None:
            operands.append(bass2jax.partition_id_tensor())
        outs = bass2jax._bass_exec_p.bind(
            *operands,
            out_avals=tuple(out_avals),
            in_names=tuple(all_in_names),
            out_names=tuple(out_names),
            lowering_input_output_aliases=(),
            sim_require_finite=True,
            sim_require_nnan=True,
            nc=nc,
        )
        return tuple(outs)

    devices = jax.devices()[:N_CORES]
    mesh = Mesh(np.asarray(devices), ("core",))
    donate = tuple(range(n_params, n_params + len(out_names)))
    sharded = jax.jit(
        shard_map(_body, mesh=mesh,
                  in_specs=(PartitionSpec("core"),) * (n_params + len(out_names)),
                  out_specs=(PartitionSpec("core"),) * len(out_names)),
        donate_argnums=donate, keep_unused=True)

    runner = {"fn": sharded, "in_names": in_names, "out_names": out_names,
              "out_avals": out_avals, "zero_shapes": zero_shapes, "nc": nc}
    _CACHE[key] = runner
    return runner


def _run(in_maps, t=T, debug_stop=None):
    r = _get_runner(t, debug_stop)
    concat_in = [
        np.concatenate([np.asarray(in_maps[c][name]) for c in range(N_CORES)],
                       axis=0)
        for name in r["in_names"]
    ]
    concat_zeros = [np.zeros((N_CORES * s[0], *s[1:]), d)
                    for (s, d) in r["zero_shapes"]]
    out_arrs = r["fn"](*concat_in, *concat_zeros)
    outs = []
    for c in range(N_CORES):
        outs.append({
            name: np.asarray(out_arrs[i]).reshape(N_CORES,
                                                  *r["out_avals"][i].shape)[c]
            for i, name in enumerate(r["out_names"])
        })
    return outs


# ---------------------------------------------------------------- entry point
def kernel(x, w_attn, w_proj):
    x = np.asarray(x, dtype=np.float32)
    w_attn = np.asarray(w_attn, dtype=np.float32)
    w_proj = np.asarray(w_proj, dtype=np.float32)
    in_maps = _prep_inputs(x, w_attn, w_proj)
    outs = _run(in_maps)
    acc = outs[0]["out"].astype(np.float32)
    for c in range(1, N_CORES):
        acc = acc + outs[c]["out"].astype(np.float32)
    return acc.reshape(B, T, E).astype(np.float32)


# revision 14
# speedup vs baseline: 1.2679x; 1.0345x over previous
"""Causal self-attention (B=1, T=2048, E=2048, 16 heads, RoPE) on 8 TRN2 NeuronCores.

Strategy: tensor-parallel over heads (2 heads/core). Each core computes
QKV for its heads, RoPE, causal softmax attention, and a PARTIAL output
projection over its 256 contraction columns of w_proj. The host sums the
8 partial [T, E] outputs (no on-device collectives).

All matmul operands are bfloat16 (fp32 PSUM accumulation, ~5e-3 max-rel
vs the 2e-2 gate). bf16 weights enable the PE's Fast Weight Load path,
halve HBM traffic, and unlock the DVE fast 16-bit modes.

Device-side layout tricks:
  - x is passed transposed: xT [E, T] so QKV matmuls contract over partitions.
  - q/k weight rows are pre-permuted per head to [even dims | odd dims] so
    RoPE becomes half-wise multiplies with cos/sin tables; the softmax scale
    1/sqrt(D) is folded into the q weights.
  - RoPE: PSUM q/k tiles are cast to bf16 once on the Scalar engine; the
    element-wise ops then run on DVE in its fast 16-bit mode.
  - scores are computed transposed: S^T[kt, qt] = kT.T @ qT, so the softmax
    sum is a ones-vector matmul and P^T feeds the V matmul directly, and
    out^T [d, qt] is exactly the lhsT the projection needs.
  - causal: only lower-triangle kt-tiles computed, and the 4 diagonal-band
    tiles are TRIMMED to their live column range [128*o, 512); one [128,128]
    0/1 triangle mask handles the残 intra-tile triangle (in-place multiply).
  - score tiles for full (non-diagonal) kt pairs share one 2-bank PSUM tile
    so exp runs once per pair (halves ACT instruction count), and the h0
    AV/row-sum matmuls are interleaved into h1's score stream so ACT's exp
    latency is hidden behind PE work.
  - normalization deferred: out^T = V^T P^T is scaled by broadcast(1/rowsum)
    past the AV matmul; 1/rowsum uses DVE reciprocal_approx_fast.
  - output partials are written in bf16 (host sums in fp32).
"""
import sys

for _p in ("/opt/trn_rl_repo",):
    if _p not in sys.path:
        sys.path.append(_p)

import numpy as np
import ml_dtypes

BF = ml_dtypes.bfloat16

B, T, E = 1, 2048, 2048
H, D = 16, 128
N_CORES = 8
HPC = H // N_CORES          # heads per core
CL = HPC * D                # contraction columns per core (256)
QC = 512                    # qt chunk (PSUM bank width in fp32)
BASE = 10000.0

_CACHE: dict = {}


# ---------------------------------------------------------------- device build
def _build_nc(t=T, debug_stop=None):
    import concourse.tile as tile
    from concourse import bacc, mybir
    from contextlib import ExitStack

    f32 = mybir.dt.float32
    bf16 = mybir.dt.bfloat16
    nj = t // QC            # qt chunks
    ntt = t // 128          # t tiles
    nct = E // 128          # contraction tiles

    nc = bacc.Bacc("TRN2", target_bir_lowering=False, debug=False,
                   enable_asserts=False, num_devices=N_CORES)
    xT_d = nc.dram_tensor("xT", [t // QC, 2, E // 256, 128, QC], bf16,
                         kind="ExternalInput").ap()
    wqkvT_d = nc.dram_tensor("wqkvT", [E, 6 * 128], bf16, kind="ExternalInput").ap()
    wprojT_d = nc.dram_tensor("wprojT", [CL, E], bf16, kind="ExternalInput").ap()
    cos2_d = nc.dram_tensor("cos2", [128, t], bf16, kind="ExternalInput").ap()
    sin2_d = nc.dram_tensor("sin2", [128, t], bf16, kind="ExternalInput").ap()
    mask_d = nc.dram_tensor("mask", [128, 128], bf16, kind="ExternalInput").ap()
    out_d = nc.dram_tensor("out", [t, E], bf16, kind="ExternalOutput").ap()

    Exp = mybir.ActivationFunctionType.Exp

    with tile.TileContext(nc) as tc:
        with ExitStack() as per:  # persistent pools
            const = per.enter_context(tc.tile_pool(name="const", bufs=1))
            wpp = per.enter_context(tc.tile_pool(name="wpp", bufs=1))
            qkp = per.enter_context(tc.tile_pool(name="qkp", bufs=1))
            vp = per.enter_context(tc.tile_pool(name="vp", bufs=1))
            atp = per.enter_context(tc.tile_pool(name="atp", bufs=1))
            ps = per.enter_context(tc.tile_pool(name="ps", bufs=1, space="PSUM"))

            # ones vectors (bf16 operands keep every matmul on the fast path)
            ones_col_f = const.tile([128, 1], f32)
            nc.vector.memset(ones_col_f[:], 1.0)
            ones_col = const.tile([128, 1], bf16)
            nc.vector.tensor_copy(ones_col[:], ones_col_f[:])
            ones_row_f = const.tile([1, 128], f32)
            nc.vector.memset(ones_row_f[:], 1.0)
            ones_row = const.tile([1, 128], bf16)
            nc.vector.tensor_copy(ones_row[:], ones_row_f[:])
            mask_sb = const.tile([128, 128], bf16)

            # persistent activations
            qk_sb = [qkp.tile([128, t], bf16, tag=f"qk{f}", name=f"qk{f}")
                     for f in range(4)]
            v_sb = vp.tile([128, ntt * 2 * D], bf16)   # [t-tile major, 2 heads*D]
            attn_sb = [atp.tile([128, t], bf16, tag=f"at{h}", name=f"at{h}")
                       for h in range(HPC)]

            # ---------------- Phase B: QKV + RoPE ----------------
            with ExitStack() as phB:
                wqp = phB.enter_context(tc.tile_pool(name="wqp", bufs=1))
                xtr = phB.enter_context(tc.tile_pool(name="xtr", bufs=4))
                rtmp = phB.enter_context(tc.tile_pool(name="rtmp", bufs=2))
                cstp = phB.enter_context(tc.tile_pool(name="cstp", bufs=1))


                # weights on the scalar-engine DMA queue, x tiles on sync,
                # constants on gpsimd -- parallel queues, compute starts asap
                wq_sb = []
                for c in range(nct):
                    w = wqp.tile([128, 6 * 128], bf16, tag=f"w{c}", name=f"w{c}")
                    eng = nc.scalar if c == 0 else nc.gpsimd
                    eng.dma_start(out=w[:], in_=wqkvT_d[c * 128:(c + 1) * 128, :])
                    wq_sb.append(w)
                # prefetch phase-C constants on the gpsimd queue (idle later)
                nc.gpsimd.dma_start(out=mask_sb[:], in_=mask_d[:])
                wp_sb = []
                for hh in range(HPC):
                    w = wpp.tile([128, E], bf16, tag=f"wp{hh}", name=f"wp{hh}")
                    nc.gpsimd.dma_start(
                        out=w[:], in_=wprojT_d[hh * 128:(hh + 1) * 128, :])
                    wp_sb.append(w)
                cos2_sb = cstp.tile([128, t], bf16)
                sin2_sb = cstp.tile([128, t], bf16)

                nhalf = nct // 2
                for j in range(nj):
                    jsl = slice(j * QC, (j + 1) * QC)
                    xts = []
                    for q in range(2):
                        xh = xtr.tile([128, nhalf * QC], bf16, tag="xt",
                                      name=f"xt{j}_{q}")
                        eng = nc.sync if q == 0 else nc.scalar
                        if j == 0:
                            # per-tile DMAs: first data reaches PE sooner
                            for cc in range(nhalf):
                                eng.dma_start(
                                    out=xh[:, cc * QC:(cc + 1) * QC],
                                    in_=xT_d[j, q, cc])
                        else:
                            eng.dma_start(
                                out=xh[:].rearrange("p (c b) -> p c b", c=nhalf),
                                in_=xT_d[j, q].rearrange("c p b -> p c b"))
                        for cc in range(nhalf):
                            xts.append(xh[:, cc * QC:(cc + 1) * QC])
                    if j == 0:
                        # rope tables after the first chunk's x tiles
                        nc.sync.dma_start(out=cos2_sb[:], in_=cos2_d[:])
                        nc.scalar.dma_start(out=sin2_sb[:], in_=sin2_d[:])
                    # q (f=0,1) and k (f=2,3), output transposed [d, t]
                    def emit_rope(f, pq):
                        # RoPE: out = pq*cos2 + swap_halves(pq)*sin2, sin2=[-sin; sin]
                        # PSUM->bf16 cast on ACT, then 16-bit DVE ops
                        pqb = rtmp.tile([128, QC], bf16, tag="pqb", name=f"pqb{j}_{f}")
                        nc.scalar.copy(pqb[:], pq[:])
                        sw = rtmp.tile([128, QC], bf16, tag="sw", name=f"sw{j}_{f}")
                        nc.vector.tensor_copy(sw[0:64, :], pqb[64:128, :])
                        nc.vector.tensor_copy(sw[64:128, :], pqb[0:64, :])
                        tB = rtmp.tile([128, QC], bf16, tag="tB", name=f"tB{j}_{f}")
                        nc.vector.tensor_mul(tB[:], sw[:], sin2_sb[:, jsl])
                        tA = rtmp.tile([128, QC], bf16, tag="tA", name=f"tA{j}_{f}")
                        nc.vector.tensor_mul(tA[:], pqb[:], cos2_sb[:, jsl])
                        nc.vector.tensor_add(qk_sb[f][:, jsl], tA[:], tB[:])

                    if j == 0:
                        # chunk 0: c-outer accumulation so matmuls start as
                        # soon as each x tile's DMA lands (4 open PSUM groups)
                        pqs = [ps.tile([128, QC], f32, tag="a", bufs=4,
                                       name=f"pq0_{f}") for f in range(4)]
                        for c in range(nct):
                            for f in range(4):
                                nc.tensor.matmul(
                                    pqs[f][:], wq_sb[c][:, f * 128:(f + 1) * 128],
                                    xts[c], start=(c == 0), stop=(c == nct - 1))
                        for f in range(4):
                            emit_rope(f, pqs[f])
                    else:
                        for f in range(4):
                            pq = ps.tile([128, QC], f32, tag="a", bufs=4,
                                         name=f"pq{j}_{f}")
                            for c in range(nct):
                                nc.tensor.matmul(
                                    pq[:], wq_sb[c][:, f * 128:(f + 1) * 128],
                                    xts[c], start=(c == 0), stop=(c == nct - 1))
                            emit_rope(f, pq)
                    # v natural [t, d] per 128-t-tile
                    for tt in range(4):
                        pv = ps.tile([128, 2 * D], f32, tag="c", bufs=3,
                                     name=f"pv{j}_{tt}")
                        for c in range(nct):
                            nc.tensor.matmul(pv[:], xts[c][:, tt * 128:(tt + 1) * 128],
                                             wq_sb[c][:, 4 * 128:6 * 128],
                                             start=(c == 0), stop=(c == nct - 1))
                        g = j * 4 + tt
                        if tt % 2 == 0:
                            nc.scalar.copy(
                                v_sb[:, g * 2 * D:(g + 1) * 2 * D], pv[:])
                        else:
                            nc.vector.tensor_copy(
                                v_sb[:, g * 2 * D:(g + 1) * 2 * D], pv[:])

            if debug_stop == "B":
                with ExitStack() as phX:
                    outp = phX.enter_context(tc.tile_pool(name="outp", bufs=2))
                    for f in range(4):
                        ob = outp.tile([128, t], bf16, tag="ob", name=f"obB{f}")
                        nc.vector.tensor_copy(ob[:], qk_sb[f][:])
                        nc.sync.dma_start(out=out_d[f * 128:(f + 1) * 128, 0:t],
                                          in_=ob[:])

            # ---------------- Phase C: causal attention ----------------
            if debug_stop not in ("B",):
                with ExitStack() as phC:
                    ptp = phC.enter_context(tc.tile_pool(name="ptp", bufs=36))
                    ctmp = phC.enter_context(tc.tile_pool(name="ctmp", bufs=3))

                    outp = phC.enter_context(tc.tile_pool(name="outp", bufs=3))

                    def emit_proj(jj, final=False):
                        # projection + output DMA for chunk jj (overlaps later
                        # attention chunks). Casts go to DVE only while ACT is
                        # busy with exp; the final chunk splits them across
                        # both engines (ACT is idle in the tail).
                        for tt in range(4 * jj, 4 * jj + 4):
                            ob = outp.tile([128, E], bf16, tag="ob", name=f"obD{tt}")
                            for oc in range(E // 512):
                                pp = ps.tile([128, 512], f32, tag="a", bufs=4,
                                             name=f"pp{tt}_{oc}")
                                for h in range(HPC):
                                    nc.tensor.matmul(
                                        pp[:], attn_sb[h][:, tt * 128:(tt + 1) * 128],
                                        wp_sb[h][:, oc * 512:(oc + 1) * 512],
                                        start=(h == 0), stop=(h == HPC - 1))
                                if final and oc % 2 == 1:
                                    nc.scalar.copy(
                                        ob[:, oc * 512:(oc + 1) * 512], pp[:])
                                else:
                                    nc.vector.tensor_copy(
                                        ob[:, oc * 512:(oc + 1) * 512], pp[:])
                            eng = nc.sync if tt % 2 == 0 else nc.scalar
                            eng.dma_start(out=out_d[tt * 128:(tt + 1) * 128, :],
                                          in_=ob[:])

                    def flush_norm(pending):
                        # deferred normalization tail: bc matmul overlaps the
                        # NEXT chunk's S^T stream instead of stalling PE
                        for (jj, h, po_, inv_) in pending:
                            jjsl = slice(jj * QC, (jj + 1) * QC)
                            invb = ctmp.tile([1, QC], bf16, tag="invb", bufs=6,
                                             name=f"invb{jj}_{h}")
                            nc.vector.tensor_copy(invb[:], inv_[:])
                            bc = ps.tile([128, QC], f32, tag="a", bufs=4,
                                         name=f"bc{jj}_{h}")
                            nc.tensor.matmul(bc[:], ones_row[:], invb[:],
                                             start=True, stop=True)
                            bcs = ctmp.tile([128, QC], bf16, tag="bcs",
                                            name=f"bcs{jj}_{h}")
                            nc.vector.tensor_copy(bcs[:], bc[:])
                            nc.vector.tensor_mul(attn_sb[h][:, jjsl], po_[:], bcs[:])

                    def emit_score(j, h, k, pts):
                        # S^T tile + exp; diagonal-band tiles trimmed to the
                        # live column range [128o, 512)
                        jq0 = j * QC
                        o = k - 4 * j
                        off = 128 * o if o > 0 else 0
                        stp = ps.tile([128, QC], f32, tag="a", bufs=4,
                                      name=f"st{j}_{h}_{k}")
                        nc.tensor.matmul(
                            stp[:, off:QC],
                            qk_sb[2 + h][:, k * 128:(k + 1) * 128],
                            qk_sb[h][:, jq0 + off:jq0 + QC],
                            start=True, stop=True)
                        pt = ptp.tile([128, QC], bf16, tag="pt",
                                      name=f"pt{j}_{h}_{k}")
                        nc.scalar.activation(pt[:, off:QC], stp[:, off:QC], Exp)
                        if o >= 0:
                            nc.vector.tensor_mul(pt[:, off:off + 128],
                                                 pt[:, off:off + 128],
                                                 mask_sb[:])
                        pts[(h, k)] = pt

                    def po_ss_step(j, h, k, po, ss, pts, nkt):
                        o = k - 4 * j
                        off = 128 * o if o > 0 else 0
                        nc.tensor.matmul(
                            po[:, off:QC],
                            v_sb[:, k * 2 * D + h * D:k * 2 * D + (h + 1) * D],
                            pts[(h, k)][:, off:QC],
                            start=(k == 0), stop=(k == nkt - 1))
                        nc.tensor.matmul(
                            ss[:, off:QC], ones_col[:],
                            pts[(h, k)][:, off:QC],
                            start=(k == 0), stop=(k == nkt - 1))

                    pending = []
                    for j in range(nj):
                        nkt = 4 * (j + 1)
                        pts = {}
                        for k in range(nkt):
                            emit_score(j, 0, k, pts)
                        # previous chunk's normalization: bc matmuls overlap
                        # this chunk's exp backlog
                        flush_norm(pending)
                        pending = []
                        # h1 scores interleaved with h0 AV/row-sum matmuls:
                        # PE spends ~640ns/tile while ACT needs ~550ns/exp,
                        # so the exp stream stays ahead of its consumers
                        po0 = ps.tile([128, QC], f32, tag="c", bufs=3,
                                      name=f"po{j}_0")
                        ss0 = ps.tile([1, QC], f32, tag="b", bufs=1,
                                      name=f"ss{j}_0")
                        for k in range(nkt):
                            emit_score(j, 1, k, pts)
                            po_ss_step(j, 0, k, po0, ss0, pts, nkt)
                        inv0 = ctmp.tile([1, QC], f32, tag="inv", bufs=6,
                                         name=f"inv{j}_0")
                        nc.vector.reciprocal_approx_fast(inv0[:], ss0[:])
                        pending.append((j, 0, po0, inv0))
                        if j > 0 and debug_stop is None:
                            # projection of the previous chunk keeps PE busy
                            # while ACT drains head-1 exps
                            emit_proj(j - 1)
                        po1 = ps.tile([128, QC], f32, tag="c", bufs=3,
                                      name=f"po{j}_1")
                        ss1 = ps.tile([1, QC], f32, tag="b", bufs=1,
                                      name=f"ss{j}_1")
                        for k in range(nkt):
                            po_ss_step(j, 1, k, po1, ss1, pts, nkt)
                        inv1 = ctmp.tile([1, QC], f32, tag="inv", bufs=6,
                                         name=f"inv{j}_1")
                        nc.vector.reciprocal_approx_fast(inv1[:], ss1[:])
                        pending.append((j, 1, po1, inv1))
                    flush_norm(pending)
                    if debug_stop is None:
                        emit_proj(nj - 1, final=True)

            if debug_stop == "C":
                with ExitStack() as phX:
                    outp = phX.enter_context(tc.tile_pool(name="outp", bufs=2))
                    for h in range(HPC):
                        ob = outp.tile([128, t], bf16, tag="ob", name=f"obC{h}")
                        nc.vector.tensor_copy(ob[:], attn_sb[h][:])
                        nc.sync.dma_start(out=out_d[h * 128:(h + 1) * 128, 0:t],
                                          in_=ob[:])


    nc.compile()
    return nc


# ---------------------------------------------------------------- host prep
def _rope_perm():
    p = np.empty(E, dtype=np.int64)
    for h in range(H):
        b = h * D
        p[b:b + 64] = b + np.arange(0, D, 2)
        p[b + 64:b + D] = b + np.arange(1, D, 2)
    return p


def _tables(t=T):
    # match reference: fp32 theta, fp32 angles; tables quantized to bf16
    theta = (1.0 / (BASE ** (np.arange(0, D, 2, dtype=np.float32) / np.float32(D)))
             ).astype(np.float32)
    m = np.arange(t, dtype=np.float32)
    fr = np.outer(m, theta).astype(np.float32)        # [t, 64]
    cos = np.cos(fr).T.astype(np.float32)             # [64, t]
    sin = np.sin(fr).T.astype(np.float32)
    cos2 = np.ascontiguousarray(np.concatenate([cos, cos], 0).astype(BF))
    sin2 = np.ascontiguousarray(np.concatenate([-sin, sin], 0).astype(BF))
    return cos2, sin2


def _mask():
    a = np.arange(128)[:, None]
    b = np.arange(128)[None, :]
    return np.ascontiguousarray((b >= a).astype(BF))


def _prep_inputs(x, w_attn, w_proj, t=T):
    x2 = np.asarray(x, dtype=np.float32).reshape(t, E)
    xT = np.ascontiguousarray(
        x2.T.reshape(2, E // 256, 128, t // QC, QC).transpose(3, 0, 1, 2, 4)
        .astype(BF))
    perm = _rope_perm()
    scale = np.float32(1.0) / np.sqrt(np.float32(D))
    wq = np.asarray(w_attn[0:E])[perm] * scale
    wk = np.asarray(w_attn[E:2 * E])[perm]
    wv = np.asarray(w_attn[2 * E:3 * E])
    cos2, sin2 = _tables(t)
    mask = _mask()
    in_maps = []
    for c in range(N_CORES):
        rows = slice(c * CL, (c + 1) * CL)
        wqkv = np.concatenate([wq[rows], wk[rows], wv[rows]], axis=0)  # [768, E]
        in_maps.append({
            "xT": xT,
            "wqkvT": np.ascontiguousarray(wqkv.T.astype(BF)),
            "wprojT": np.ascontiguousarray(
                np.asarray(w_proj)[:, rows].T.astype(BF)),
            "cos2": cos2,
            "sin2": sin2,
            "mask": mask,
        })
    return in_maps


# ---------------------------------------------------------------- cached runner
def _get_runner(t=T, debug_stop=None):
    """Build the Bass module once and return a cached jitted executor.

    Mirrors concourse.bass2jax.run_bass_via_pjrt's multi-core branch, but
    keeps the jitted callable so repeated kernel() calls don't recompile.
    """
    key = ("runner", t, debug_stop)
    if key in _CACHE:
        return _CACHE[key]
    import jax
    from concourse import bass2jax, mybir
    from jax.experimental.shard_map import shard_map
    from jax.sharding import Mesh, PartitionSpec

    nc = _build_nc(t, debug_stop)
    bass2jax.install_neuronx_cc_hook()

    partition_name = (nc.partition_id_tensor.name if nc.partition_id_tensor
                      else None)
    in_names, out_names, out_avals, zero_shapes = [], [], [], []
    for alloc in nc.m.functions[0].allocations:
        if not isinstance(alloc, mybir.MemoryLocationSet):
            continue
        name = alloc.memorylocations[0].name
        if alloc.kind == "ExternalInput":
            if name != partition_name:
                in_names.append(name)
        elif alloc.kind == "ExternalOutput":
            shape = tuple(alloc.tensor_shape)
            dtype = mybir.dt.np(alloc.dtype)
            out_names.append(name)
            out_avals.append(jax.core.ShapedArray(shape, dtype))
            zero_shapes.append((shape, dtype))
    n_params = len(in_names)
    all_in_names = list(in_names) + list(out_names)
    if partition_name is not None:
        all_in_names.append(partition_name)

    def _body(*args):
        operands = list(args)
        if partition_name is not None:
            operands.append(bass2jax.partition_id_tensor())
        outs = bass2jax._bass_exec_p.bind(
            *operands,
            out_avals=tuple(out_avals),
            in_names=tuple(all_in_names),
            out_names=tuple(out_names),
            lowering_input_output_aliases=(),
            sim_require_finite=True,
            sim_require_nnan=True,
            nc=nc,
        )
        return tuple(outs)

    devices = jax.devices()[:N_CORES]
    mesh = Mesh(np.asarray(devices), ("core",))
    donate = tuple(range(n_params, n_params + len(out_names)))
    sharded = jax.jit(
        shard_map(_body, mesh=mesh,
                  in_specs=(PartitionSpec("core"),) * (n_params + len(out_names)),
                  out_specs=(PartitionSpec("core"),) * len(out_names)),
        donate_argnums=donate, keep_unused=True)

    runner = {"fn": sharded, "in_names": in_names, "out_names": out_names,
              "out_avals": out_avals, "zero_shapes": zero_shapes, "nc": nc}
    _CACHE[key] = runner
    return runner


def _run(in_maps, t=T, debug_stop=None):
    r = _get_runner(t, debug_stop)
    concat_in = [
        np.concatenate([np.asarray(in_maps[c][name]) for c in range(N_CORES)],
                       axis=0)
        for name in r["in_names"]
    ]
    concat_zeros = [np.zeros((N_CORES * s[0], *s[1:]), d)
                    for (s, d) in r["zero_shapes"]]
    out_arrs = r["fn"](*concat_in, *concat_zeros)
    outs = []
    for c in range(N_CORES):
        outs.append({
            name: np.asarray(out_arrs[i]).reshape(N_CORES,
                                                  *r["out_avals"][i].shape)[c]
            for i, name in enumerate(r["out_names"])
        })
    return outs


# ---------------------------------------------------------------- entry point
def kernel(x, w_attn, w_proj):
    x = np.asarray(x, dtype=np.float32)
    w_attn = np.asarray(w_attn, dtype=np.float32)
    w_proj = np.asarray(w_proj, dtype=np.float32)
    in_maps = _prep_inputs(x, w_attn, w_proj)
    outs = _run(in_maps)
    acc = outs[0]["out"].astype(np.float32)
    for c in range(1, N_CORES):
        acc = acc + outs[c]["out"].astype(np.float32)
    return acc.reshape(B, T, E).astype(np.float32)
